# revision 20
# baseline (speedup 1.0000x reference)
"""LocallyConnected1D (B=8, L=4096, C=64, K=3, F=64) on 8 TRN2 NeuronCores.

out[b, l, f] = sum_{k,c} x[b, l+k, c] * kernel[l, k, c, f] + bias[l, f]

Strategy (spatial sharding, 512 output positions per core):
  - For each pair of adjacent output positions (l0+2i, l0+2i+1) build a
    block-diagonal stationary tile lhsT (128 x 16): partitions = 2 phases x 64
    channels, columns = 2 phases x 8 batch.  Streaming operand = the pair's
    per-position weights (128 x 64).  Three PSUM-accumulated matmuls per pair
    (one per tap k, using x-pair tiles shifted by k) produce out (16, 64).
  - Groups of 8 pairs are dispatched to 4 independent 32-column strips of the
    PE array (tile_position), each strip accumulating into its own PSUM bank,
    so up to 4 matmuls run concurrently in the array.
  - Weights AND x-pair tiles are packed into one contiguous DRAM blob per
    block -> dense DMAs at full HBM bandwidth.  First blocks are small so the
    PE starts early; per-block outputs go out in a single DMA.
  - Compute in bf16 (PSUM accumulation in f32); bias added on host.
"""

import numpy as np
import ml_dtypes

import concourse.bass as bass
import concourse.mybir as mybir
import concourse.tile as tile
from concourse import bacc
from concourse.bass import ds, ts
from concourse.bass_utils import run_bass_kernel_spmd

B, L, C, K, F = 8, 4096, 64, 3, 64
L_OUT = (L - K) + 1  # 4094
N_CORES = 8
P_CORE = 512          # output positions per core (last core: 510 real + 2 pad)
PAIRS = P_CORE // 2   # 256

# pairs per DMA block; small first blocks let the PE start early
BLOCKS = [8, 8] + [16] * 14 + [8, 8]
assert sum(BLOCKS) == PAIRS and all(b % 8 == 0 for b in BLOCKS)

USE_BF16 = True
DT = mybir.dt.bfloat16 if USE_BF16 else mybir.dt.float32
NPDT = ml_dtypes.bfloat16 if USE_BF16 else np.float32
DT_OUT = mybir.dt.float32

# per-block columns (per partition): weights | te tiles | to tiles
def _blk_cols(n):
    return n * K * F + (n + 1) * 16 + n * 16

BLK_OFF = np.cumsum([0] + [_blk_cols(n) for n in BLOCKS]).tolist()
TOT_COLS = BLK_OFF[-1]

_CACHE = {}


def _build_body(nc, wpool, opool, pspool, blk_d, out_d):
    s = 0  # first pair of current block
    for h, n in enumerate(BLOCKS):
        cols = _blk_cols(n)
        blk = wpool.tile([128, cols], DT, name="blk", tag="blk",
                         padded_shape=[128, _blk_cols(max(BLOCKS))])
        nc.sync.dma_start(blk[:], blk_d[:, ds(BLK_OFF[h], cols)])
        w_cols = n * K * F
        te_cols = (n + 1) * 16
        ngroups = n // 8
        accs = [pspool.tile([128, 512], DT_OUT, name=f"acc{q}", tag=f"acc{q}")
                for q in range(ngroups)]

        def te_ap(i):   # block-diag tile for even-start pair i
            return blk[:, ds(w_cols + (i - s) * 16, 16)]

        def to_ap(i):   # odd-start pair i
            return blk[:, ds(w_cols + te_cols + (i - s) * 16, 16)]

        def w_ap(jj, k):
            return blk[:, ds((jj * K + k) * F, F)]

        for j in range(8):
            for q in range(ngroups):
                i = s + q * 8 + j   # global pair
                jj = q * 8 + j      # pair in block
                o_ap = accs[q][ds(32 * q, 16), ts(j, 64)]
                tp = (0, 32 * q)
                nc.tensor.matmul(o_ap, te_ap(i), w_ap(jj, 0),
                                 start=True, stop=False, tile_position=tp)
                nc.tensor.matmul(o_ap, to_ap(i), w_ap(jj, 1),
                                 start=False, stop=False, tile_position=tp)
                nc.tensor.matmul(o_ap, te_ap(i + 1), w_ap(jj, 2),
                                 start=False, stop=True, tile_position=tp)
        ob = opool.tile([16, ngroups * 512], DT_OUT, name="ob", tag="ob",
                        padded_shape=[16, 4 * 512])
        for q in range(ngroups):
            nc.vector.tensor_copy(ob[:, ds(q * 512, 512)],
                                  accs[q][ds(32 * q, 16), :])
        g0 = s // 8  # first global group of this block
        nc.scalar.dma_start(out_d[:, ds(g0 * 512, ngroups * 512)], ob[:])
        s += n


def _build_nc(n_iters=None):
    """n_iters=None: straight-line kernel (graded path).
    n_iters=N: body wrapped in a HW For_i loop, for timing-slope runs."""
    nc = bacc.Bacc("TRN2", target_bir_lowering=False, debug=False)

    blk_d = nc.declare_dram_parameter("blk", [128, TOT_COLS], DT, isOutput=False)
    # out[m, g*512 + j*64 + f]: g = group of 8 pairs, m = phase*8 + b.
    out_d = nc.declare_dram_parameter("out", [16, (PAIRS // 8) * 512], DT_OUT,
                                      isOutput=True)

    with tile.TileContext(nc) as tc:
        with (
            tc.tile_pool(name="wpool", bufs=8) as wpool,
            tc.tile_pool(name="opool", bufs=8) as opool,
            # 4 acc tags (one per PE strip) x 2 bufs = all 8 PSUM banks
            tc.tile_pool(name="pspool", bufs=2, space=bass.MemorySpace.PSUM) as pspool,
        ):
            if n_iters is None:
                _build_body(nc, wpool, opool, pspool, blk_d, out_d)
            else:
                with tc.For_i(0, n_iters, 1):
                    _build_body(nc, wpool, opool, pspool, blk_d, out_d)

    nc.compile()
    return nc


def _prep_inputs(x, kernel):
    """Host-side rearrangement into per-core fused block layouts."""
    xp = np.zeros((B, L + 4, C), np.float32)
    xp[:, :L] = x
    kp = np.zeros((N_CORES * P_CORE, K, C, F), np.float32)
    kp[:L_OUT] = kernel
    in_maps = []
    for m in range(N_CORES):
        l0 = P_CORE * m
        xs = xp[:, l0:l0 + 2 * PAIRS + 2, :]
        ev = xs[:, 0::2].transpose(2, 1, 0)  # (64, 257, 8)  j = 2i
        od = xs[:, 1::2].transpose(2, 1, 0)  # (64, 257, 8)  j = 2i+1
        # TE[i]: pair (2i, 2i+1); TO[i]: pair (2i+1, 2i+2); block-diag (128,16)
        TE = np.zeros((128, PAIRS + 1, 16), np.float32)
        TE[:64, :, 0:8] = ev
        TE[64:, :, 8:16] = od
        TO = np.zeros((128, PAIRS, 16), np.float32)
        TO[:64, :, 0:8] = od[:, :PAIRS]
        TO[64:, :, 8:16] = ev[:, 1:PAIRS + 1]
        W = (kp[l0:l0 + P_CORE]
             .reshape(PAIRS, 2, K, C, F)
             .transpose(1, 3, 0, 2, 4)
             .reshape(128, PAIRS, K, F))  # [pc, pair, k, f]
        blk = np.empty((128, TOT_COLS), np.float32)
        s = 0
        for h, n in enumerate(BLOCKS):
            o = BLK_OFF[h]
            w_cols = n * K * F
            blk[:, o:o + w_cols] = W[:, s:s + n].reshape(128, w_cols)
            blk[:, o + w_cols:o + w_cols + (n + 1) * 16] = (
                TE[:, s:s + n + 1].reshape(128, (n + 1) * 16))
            blk[:, o + w_cols + (n + 1) * 16:o + _blk_cols(n)] = (
                TO[:, s:s + n].reshape(128, n * 16))
            s += n
        in_maps.append({"blk": blk.astype(NPDT)})
    return in_maps


def _unpack_out(res):
    """(16, 32*512) per core -> (B, P_CORE, F).  l_local = 16g + 2j + phase."""
    return (res.reshape(2, 8, 32, 8, 64)          # [phase, b, g, j, f]
            .transpose(1, 2, 3, 0, 4)              # [b, g, j, phase, f]
            .reshape(B, P_CORE, F))


def kernel(x, kernel, bias):
    x = np.asarray(x, dtype=np.float32)
    kern = np.asarray(kernel, dtype=np.float32)
    bias = np.asarray(bias, dtype=np.float32)

    if "nc" not in _CACHE:
        _CACHE["nc"] = _build_nc()
    nc = _CACHE["nc"]

    in_maps = _prep_inputs(x, kern)
    results = run_bass_kernel_spmd(nc, in_maps, list(range(N_CORES))).results

    parts = [_unpack_out(results[m]["out"]) for m in range(N_CORES)]
    out = np.concatenate(parts, axis=1)[:, :L_OUT]
    return (out + bias[None]).astype(np.float32)
